# revision 7
# baseline (speedup 1.0000x reference)
"""Trainium2 Bass kernel for the word2vec-style embedding_lookup problem.

reference math (per row b of data [B, 22], all f32):
  ctx_idx  = data[:, :10]    (into global_W [100001, 128])
  pos_idx  = data[:, 11]     (into sense_W  [300000, 128])
  neg_idx  = data[:, 12:17]  (into sense_W)
  mask     = data[:, 17:22]  (float multiplier for neg loss)
  ctx_feats = sum_j global_W[ctx_idx[:, j]] * ctx_weight[j]          # [B, 128]
  pos_ips   = dot(ctx_feats, sense_W[pos_idx])                        # [B]
  pos_loss  = sum(softplus(-clip(pos_ips, -10, 10)))
  neg_ips   = dot(ctx_feats, sense_W[neg_idx[:, n]])                  # [B, 5]
  neg_loss  = sum(softplus(clip(neg_ips, -10, 10)) * mask)

Sharding: data-parallel over 8 NeuronCores, 16384 rows each; the two
embedding tables are concatenated into one [400001, 128] table replicated
to every core.  Each core returns its two partial losses; the host sums.

Device mapping per 128-row block:
  - one gpsimd indirect DMA gathers the 16 embedding rows of each of the
    128 data rows into an SBUF tile [128, 16*128]
  - DVE: multiply ctx part by (pre-broadcast) ctx_weight, strided reduce
    over the 10 context slots, multiply sense part by broadcast ctx_feats
  - ACT: per-slot accumulate (dot products), then the clip+softplus chain
    via relu(x+10) -> relu(20-x) -> softplus(+/-(x-10))
  - per-block results land in slot buffers; one final reduce + PE
    ones-matmul collapses partitions to the two scalar losses.
"""

import numpy as np

V = 100000
D = 128
NCTX = 10  # 2*window
NSNS = 6   # 1 pos + 5 neg
K = NCTX + NSNS
B = 131072
NCORES = 8
BCORE = B // NCORES
NBLK_FULL = BCORE // 128
SENSE_OFF = V + 1
TABLE_ROWS_FULL = (V + 1) + 3 * V

_cache = {}


def build_nc(nblk, table_rows, debug_outs=False):
    """Build and compile the per-core Bass program."""
    import concourse.bacc as bacc
    import concourse.bass as bass
    import concourse.mybir as mybir
    import concourse.tile as tile

    f32 = mybir.dt.float32
    i32 = mybir.dt.int32
    ALU = mybir.AluOpType
    ACTF = mybir.ActivationFunctionType
    AX = mybir.AxisListType

    nc = bacc.Bacc("TRN2", target_bir_lowering=False, debug=False)

    table = nc.dram_tensor("table", [table_rows, D], f32, kind="ExternalInput")
    idx = nc.dram_tensor("idx", [128, nblk * K], i32, kind="ExternalInput")
    msk = nc.dram_tensor("msk", [128, nblk * NSNS], f32, kind="ExternalInput")
    wb = nc.dram_tensor("wb", [128, NCTX * D], f32, kind="ExternalInput")
    out = nc.dram_tensor("out", [1, 2], f32, kind="ExternalOutput")
    if debug_outs:
        d_ips = nc.dram_tensor("d_ips", [128, nblk * NSNS], f32, kind="ExternalOutput")
        d_u = nc.dram_tensor("d_u", [128, nblk * NSNS], f32, kind="ExternalOutput")
        d_g = nc.dram_tensor("d_g", [128, K * D], f32, kind="ExternalOutput")
        d_F = nc.dram_tensor("d_F", [128, D], f32, kind="ExternalOutput")
        d_bufP = nc.dram_tensor("d_bufP", [128, nblk], f32, kind="ExternalOutput")
        d_bufN = nc.dram_tensor("d_bufN", [128, nblk * 5], f32, kind="ExternalOutput")

    with tile.TileContext(nc) as tc:
        with (
            tc.tile_pool(name="const", bufs=1) as constp,
            tc.tile_pool(name="gpool", bufs=4) as gp,
            tc.tile_pool(name="wpool", bufs=2) as wp,
            tc.tile_pool(name="spool", bufs=2) as sp,
            tc.tile_pool(name="small", bufs=2) as smp,
            tc.tile_pool(name="psum", bufs=1, space="PSUM") as psp,
        ):
            idx_t = constp.tile([128, nblk * K], i32)
            nc.sync.dma_start(out=idx_t[:], in_=idx[:])
            msk_t = constp.tile([128, nblk * NSNS], f32)
            nc.sync.dma_start(out=msk_t[:], in_=msk[:])
            wb_t = constp.tile([128, NCTX * D], f32)
            nc.sync.dma_start(out=wb_t[:], in_=wb[:])

            bufP = constp.tile([128, nblk], f32)
            bufN = constp.tile([128, nblk * 5], f32)
            dummy = constp.tile([128, D], f32)
            ones = constp.tile([128, 1], f32)
            nc.vector.memset(ones[:], 1.0)
            c10 = constp.tile([128, 1], f32)
            nc.vector.memset(c10[:], 10.0)
            c20 = constp.tile([128, 1], f32)
            nc.vector.memset(c20[:], 20.0)
            cm10 = constp.tile([128, 1], f32)
            nc.vector.memset(cm10[:], -10.0)

            for b in range(nblk):
                g = gp.tile([128, K * D], f32, tag="g")
                # HW vector-indirect DMA consumes ONE offset per partition
                # per instruction -> 16 gathers of [128, D] per block.
                for k in range(K):
                    nc.gpsimd.indirect_dma_start(
                        out=g[:, k * D : (k + 1) * D],
                        out_offset=None,
                        in_=table[:],
                        in_offset=bass.IndirectOffsetOnAxis(
                            ap=idx_t[:, b * K + k : b * K + k + 1], axis=0
                        ),
                    )
                # ctx part * ctx_weight
                wprod = wp.tile([128, NCTX * D], f32, tag="wprod")
                nc.vector.tensor_tensor(
                    out=wprod[:], in0=g[:, : NCTX * D], in1=wb_t[:], op=ALU.mult
                )
                # ctx_feats: reduce over the 10 ctx slots (strided view)
                F = smp.tile([128, D], f32, tag="F")
                nc.vector.tensor_reduce(
                    out=F[:],
                    in_=wprod[:].rearrange("p (j d) -> p d j", j=NCTX),
                    axis=AX.X,
                    op=ALU.add,
                )
                # sense part * broadcast ctx_feats
                S = sp.tile([128, NSNS * D], f32, tag="S")
                nc.vector.tensor_tensor(
                    out=S[:].rearrange("p (n d) -> p n d", n=NSNS),
                    in0=g[:, NCTX * D :].rearrange("p (n d) -> p n d", n=NSNS),
                    in1=F[:].unsqueeze(1).to_broadcast([128, NSNS, D]),
                    op=ALU.mult,
                )
                # dot products: per-slot free-dim accumulate on ACT
                ips = smp.tile([128, NSNS], f32, tag="ips")
                for n in range(NSNS):
                    nc.scalar.activation(
                        out=dummy[:],
                        in_=S[:, n * D : (n + 1) * D],
                        func=ACTF.Copy,
                        accum_out=ips[:, n : n + 1],
                    )
                # clip+softplus chain:
                #   t = relu(ips + 10); u = relu(20 - t)  (u = 10 - clip(ips))
                #   pos elem = softplus(u0 - 10);  neg elem = softplus(10 - u)
                t1 = smp.tile([128, NSNS], f32, tag="t1")
                nc.scalar.activation(
                    out=t1[:], in_=ips[:], func=ACTF.Relu, bias=c10[:], scale=1.0
                )
                u = smp.tile([128, NSNS], f32, tag="u")
                nc.scalar.activation(
                    out=u[:], in_=t1[:], func=ACTF.Relu, bias=c20[:], scale=-1.0
                )
                # softplus(x) = Ln(exp(x) + 1); pos x = u0 - 10, neg x = 10 - u
                ep = smp.tile([128, 1], f32, tag="ep")
                nc.scalar.activation(
                    out=ep[:], in_=u[:, 0:1], func=ACTF.Exp, bias=cm10[:], scale=1.0
                )
                nc.scalar.activation(
                    out=bufP[:, b : b + 1], in_=ep[:], func=ACTF.Ln, bias=1.0, scale=1.0
                )
                en = smp.tile([128, 5], f32, tag="en")
                nc.scalar.activation(
                    out=en[:], in_=u[:, 1:NSNS], func=ACTF.Exp, bias=c10[:], scale=-1.0
                )
                Ln = smp.tile([128, 5], f32, tag="Ln")
                nc.scalar.activation(
                    out=Ln[:], in_=en[:], func=ACTF.Ln, bias=1.0, scale=1.0
                )
                nc.vector.tensor_tensor(
                    out=bufN[:, b * 5 : (b + 1) * 5],
                    in0=Ln[:],
                    in1=msk_t[:, b * NSNS + 1 : (b + 1) * NSNS],
                    op=ALU.mult,
                )
                if debug_outs:
                    nc.sync.dma_start(
                        out=d_ips[:, b * NSNS : (b + 1) * NSNS], in_=ips[:]
                    )
                    nc.sync.dma_start(out=d_u[:, b * NSNS : (b + 1) * NSNS], in_=u[:])
                    if b == 0:
                        nc.sync.dma_start(out=d_g[:], in_=g[:])
                        nc.sync.dma_start(out=d_F[:], in_=F[:])

            if debug_outs:
                nc.sync.dma_start(out=d_bufP[:], in_=bufP[:])
                nc.sync.dma_start(out=d_bufN[:], in_=bufN[:])
            acc2 = constp.tile([128, 2], f32)
            nc.vector.tensor_reduce(
                out=acc2[:, 0:1], in_=bufP[:], axis=AX.X, op=ALU.add
            )
            nc.vector.tensor_reduce(
                out=acc2[:, 1:2], in_=bufN[:], axis=AX.X, op=ALU.add
            )
            ps = psp.tile([1, 2], f32)
            nc.tensor.matmul(out=ps[:], lhsT=ones[:], rhs=acc2[:], start=True, stop=True)
            fin = smp.tile([1, 2], f32, tag="fin")
            nc.vector.tensor_copy(out=fin[:], in_=ps[:])
            nc.sync.dma_start(out=out[:], in_=fin[:])

    nc.compile()
    return nc


def get_nc(nblk, table_rows):
    key = (nblk, table_rows)
    if key not in _cache:
        _cache[key] = build_nc(nblk, table_rows)
    return _cache[key]


def host_prep(data, global_W, sense_W, ctx_weight, ncores, nblk):
    """Shard + lay out the inputs for the per-core kernel."""
    data = np.asarray(data)
    global_W = np.asarray(global_W, dtype=np.float32)
    sense_W = np.asarray(sense_W, dtype=np.float32)
    ctx_weight = np.asarray(ctx_weight, dtype=np.float32)

    b = data.shape[0]
    bcore = b // ncores
    assert bcore == nblk * 128

    idx_all = np.empty((b, K), dtype=np.int32)
    idx_all[:, :NCTX] = data[:, :NCTX]
    idx_all[:, NCTX] = data[:, NCTX + 1] + SENSE_OFF
    idx_all[:, NCTX + 1 :] = data[:, NCTX + 2 : NCTX + 7] + SENSE_OFF

    msk_all = np.empty((b, NSNS), dtype=np.float32)
    msk_all[:, 0] = 1.0
    msk_all[:, 1:] = data[:, NCTX + 7 :].astype(np.float32)

    table = np.ascontiguousarray(
        np.concatenate([global_W, sense_W], axis=0), dtype=np.float32
    )
    wb = np.ascontiguousarray(
        np.broadcast_to(ctx_weight.reshape(1, NCTX * D), (128, NCTX * D)),
        dtype=np.float32,
    )

    in_maps = []
    for c in range(ncores):
        sl = slice(c * bcore, (c + 1) * bcore)
        idx_c = np.ascontiguousarray(
            idx_all[sl].reshape(nblk, 128, K).transpose(1, 0, 2).reshape(128, nblk * K)
        )
        msk_c = np.ascontiguousarray(
            msk_all[sl]
            .reshape(nblk, 128, NSNS)
            .transpose(1, 0, 2)
            .reshape(128, nblk * NSNS)
        )
        in_maps.append({"table": table, "idx": idx_c, "msk": msk_c, "wb": wb})
    return in_maps


def kernel(data, global_W, sense_W, ctx_weight, window, negative):
    from concourse.bass_utils import run_bass_kernel_spmd

    assert int(window) == 5 and int(negative) == 5

    nc = get_nc(NBLK_FULL, TABLE_ROWS_FULL)
    in_maps = host_prep(data, global_W, sense_W, ctx_weight, NCORES, NBLK_FULL)
    res = run_bass_kernel_spmd(nc, in_maps, core_ids=list(range(NCORES)))
    outs = np.stack([r["out"][0] for r in res.results])  # [ncores, 2]
    tot = outs.sum(axis=0)
    return (np.float32(tot[0]), np.float32(tot[1]))
